# revision 52
# baseline (speedup 1.0000x reference)
"""PostCrossAttention Trainium2 kernel.

Reference computation (per batch b):
    qh = (q @ Wq.T)  split into H=8 heads of dh=96   -> [H, N, 96]
    kh = (k @ Wk.T)  likewise
    vh = (v @ Wv.T)  split into H=8 heads of dv=64   -> [H, N, 64]
    S  = qh @ kh.T * SCALE          (SCALE = (256//8)**-0.5 = 32**-0.5)
    A  = softmax(S, axis=-1)
    A  = A * m / (H * sum(m, -1, keepdims))
    x  = A @ vh   -> concat heads -> [N, 512]

Sharding: 8 cores = 4 batches x 2 head-groups (4 heads each).

Everything is bf16 on device. (fp8 was measured numerically dead here:
x is a weighted mean of zero-mean v, so quantization noise on the
attention weights or on v does NOT average down — output rel err ~=
input rel err ~= 10% for e4m3, vs the 2e-2 gate.)

Device computes, per core and head, U.T[dv, i] = sum_j expS[j,i]*m[j,i]*V[j,dv]
plus running exp-sum accumulators. The final x = U / (sumexp * 8 * summ)
division, the U.T transpose, the mask row sums (summ) and the sumexp
partition contraction all happen on the host — they cost nothing there
and free the device of the transpose matmuls, the summ phase, the
sumexp ones-matmuls, and the epilogue DVE ops.

Device dataflow (per core, per head, per i-half), j-tiles processed in
PAIRS so each elementwise instruction covers 2 tiles (halved overhead):
    S.T[j,i] = Kp @ Qp.T       (PE, lhsT=KpT tile, rhs=QpT, K=96)
    expS.T   = exp(S.T * SCALE)        (ACT, per j-tile, into pair tile)
    B.T      = expS.T * masks.T        (DVE per pair)
    eacc    += expS.T                  (DVE, pairs 0..5; pairs 6,7 ship raw)
    U.T[0:64 ,i] += Vp[jt].T @ B.T[jt] (PE, deferred one pair behind S)
    out  <- U.T[0:64, i]               (ACT copy PSUM->SBUF, DMA out)
    eout <- eacc, expS.T[pairs 6,7]    (DMA; host sums partitions)

Scheduling notes (these were each worth 20-80us on HW):
  - GpSimd is ~8x slower per element than the DVE for TensorTensor/Copy
    and cannot read PSUM; it is used ONLY to issue gated mask DMAs.
  - Every phase's tail (last AV pair, output copy + DMAs) is deferred
    into the next phase so the in-order engine queues never block at a
    phase boundary; the next phase's first S matmuls go first of all.
  - DMA transfers stripe across all 16 engines concurrently, so issue
    order alone is no priority: k/v/mask loads are held back by
    WAW corner-writes gated on projection outputs, giving the q load
    (which gates everything) the full bandwidth at t=0.
  - The scheduler has a small out-of-order window, so each gated DMA
    needs its OWN dependency, not one shared gate at the front.
"""

import sys

for _p in ("/opt/trn_rl_repo",):
    if _p not in sys.path:
        sys.path.insert(0, _p)

from contextlib import ExitStack

import ml_dtypes
import numpy as np

import concourse.bass as bass
import concourse.bacc as bacc_mod
import concourse.mybir as mybir
import concourse.tile as tile

F32 = mybir.dt.float32
BF16 = mybir.dt.bfloat16
BF16NP = ml_dtypes.bfloat16

# Problem constants (hardcoded per harness contract)
B, N, C, CV, H = 4, 2048, 768, 512, 8
DH, DV = C // H, CV // H          # 96, 64
NH = 4                            # heads per core
NDO = NH * DH                     # 384 projected q/k dims per core
NDV = NH * DV                     # 256 projected v dims per core
SCALE = float((256 // 8) ** (-0.5))
N_CORES = 8

RAW_PAIRS = 4                     # trailing j-tile pairs shipped unsummed


def build_nc(NT: int = N):
    """Build the per-core Bass program. NT = token count (param for small sims)."""
    NJT = NT // 128               # j tiles
    NPR = NJT // 2                # j-tile pairs
    assert NT % 512 == 0 and NJT % 2 == 0

    NCT = C // 128                # 6 c tiles
    NVT = CV // 128               # 4 cv tiles
    WALL = 2 * NCT * NDO + NVT * NDV
    IH = min(1024, NT)            # i-half width
    NHF = NT // IH                # number of i-halves
    NRW = min(RAW_PAIRS, NPR - 1) # raw-shipped pairs
    ESH = (1 + NRW) * 2 * IH      # exp-sum free elems shipped per phase

    nc = bacc_mod.Bacc()
    # all inputs host-packed to the exact SBUF image: [128, k*W] where
    # partition p row-interleaves rows {p, 128+p, ...} of the logical tensor
    qT = nc.declare_dram_parameter("qT", [128, NCT * NT], BF16, isOutput=False)
    kT = nc.declare_dram_parameter("kT", [128, NCT * NT], BF16, isOutput=False)
    vT = nc.declare_dram_parameter("vT", [128, NVT * NT], BF16, isOutput=False)
    mT = nc.declare_dram_parameter("mT", [128, NJT * NT], BF16, isOutput=False)
    wall = nc.declare_dram_parameter("wall", [128, WALL], BF16, isOutput=False)
    # U.T rows 0..63 per (head, i-half); raw exp-sum pairs ship separately
    out = nc.declare_dram_parameter("out", [DV, NH * NHF * IH], F32,
                                    isOutput=True)
    eout = nc.declare_dram_parameter("eout", [128, NH * NHF * ESH], BF16,
                                     isOutput=True)

    with ExitStack() as top:
        tc = top.enter_context(tile.TileContext(nc))
        persist = top.enter_context(tc.tile_pool(name="persist", bufs=1))

        mt_all = persist.tile([128, NJT, NT], BF16, tag="mt", name="mt_all")

        # ---- projections ----
        qpt = [persist.tile([DH, NT], BF16, tag=f"qpt{h}", name=f"qpt{h}") for h in range(NH)]
        kpt = [persist.tile([DH, NT], BF16, tag=f"kpt{h}", name=f"kpt{h}") for h in range(NH)]
        vp = persist.tile([128, NJT, NDV], BF16, tag="vp", name="vp")

        with ExitStack() as projctx:
            qkv_pool = projctx.enter_context(tc.tile_pool(name="qkv", bufs=1))
            w_pool = projctx.enter_context(tc.tile_pool(name="w", bufs=1))
            ppsum = projctx.enter_context(
                tc.tile_pool(name="ppsum", bufs=4, space="PSUM"))
            PP_BUFS, PV_BUFS = 6, 2   # 6+2 PSUM banks during projection

            wq_v = w_pool.tile([128, NCT, NDO], BF16, tag="wq", name="wq_sb")
            wk_v = w_pool.tile([128, NCT, NDO], BF16, tag="wk", name="wk_sb")
            wv_v = w_pool.tile([128, NVT, NDV], BF16, tag="wv", name="wv_sb")
            q_sb = qkv_pool.tile([128, NCT, NT], BF16, tag="q", name="q")
            k_sb = qkv_pool.tile([128, NCT, NT], BF16, tag="k", name="k")
            v_sb = qkv_pool.tile([128, NVT, NT], BF16, tag="v", name="v")
            gates = qkv_pool.tile([8, 8], BF16, tag="gate", name="gates")

            def load_chunks(dst, dram, n_tiles, width, fracs):
                w2 = n_tiles * width
                edges = [0] + list(np.cumsum(fracs))
                tot = edges[-1]
                for s in range(len(fracs)):
                    a = edges[s] * w2 // tot
                    b = edges[s + 1] * w2 // tot
                    nc.sync.dma_start(
                        out=dst.rearrange("p a n -> p (a n)")[:, a:b],
                        in_=dram[:, a:b])

            def gate_on(src_ap, gi, corner_aps):
                """Write a gated byte into each corner so the DMAs that
                cover those regions wait for src_ap's producer."""
                nc.gpsimd.tensor_copy(out=gates[0:1, gi:gi + 1], in_=src_ap)
                for ca in corner_aps:
                    nc.gpsimd.tensor_copy(
                        out=ca, in_=gates[0:1, gi:gi + 1])

            # q (and its weights) get the full DMA bandwidth first
            nc.sync.dma_start(
                out=wq_v.rearrange("p a n -> p (a n)"),
                in_=wall[:, 0:NCT * NDO])
            load_chunks(q_sb, qT, NCT, NT, fracs=[1, 1, 2, 2])

            NCH = NT // 512

            def qk_head(dst, wv_, xv, h):
                pss = [ppsum.tile([DH, 512], F32, tag="pp", name="pp",
                                  bufs=PP_BUFS) for _ in range(NCH)]
                for ci in range(NCT):
                    for ch in range(NCH):
                        nc.tensor.matmul(
                            pss[ch],
                            lhsT=wv_[:, ci, h * DH:(h + 1) * DH],
                            rhs=xv[:, ci, ch * 512:(ch + 1) * 512],
                            start=(ci == 0), stop=(ci == NCT - 1),
                        )
                for ch in range(NCH):
                    nc.vector.tensor_copy(
                        out=dst[h][:, ch * 512:(ch + 1) * 512],
                        in_=pss[ch])

            # head 0 of q, then release the k load (gated on its output)
            qk_head(qpt, wq_v, q_sb, 0)
            gate_on(qpt[0][0:1, 0:1], 0,
                    [wk_v[0:1, 0:1, 0:1]]
                    + [k_sb[0:1, 2 * s:2 * s + 1, 0:1] for s in range(3)])
            nc.sync.dma_start(
                out=wk_v.rearrange("p a n -> p (a n)"),
                in_=wall[:, NCT * NDO:2 * NCT * NDO])
            load_chunks(k_sb, kT, NCT, NT, fracs=[2, 2, 2])

            qk_head(qpt, wq_v, q_sb, 1)
            # release the v load
            gate_on(qpt[1][0:1, 0:1], 1,
                    [wv_v[0:1, 0:1, 0:1], v_sb[0:1, 0:1, 0:1]])
            nc.sync.dma_start(
                out=wv_v.rearrange("p a n -> p (a n)"),
                in_=wall[:, 2 * NCT * NDO:])
            load_chunks(v_sb, vT, NVT, NT, fracs=[4])

            qk_head(qpt, wq_v, q_sb, 2)
            # release the mask load (needed only from attention start)
            nc.gpsimd.tensor_copy(out=gates[0:1, 2:3], in_=qpt[2][0:1, 0:1])
            for s in range(8):
                a, b = s * NJT // 8, (s + 1) * NJT // 8
                nc.gpsimd.tensor_copy(
                    out=mt_all[0:1, a:a + 1, 0:1], in_=gates[0:1, 2:3])
                nc.gpsimd.dma_start(
                    out=mt_all[:, a:b, :],
                    in_=mT[:, a * NT:b * NT])
            qk_head(qpt, wq_v, q_sb, 3)

            for h in range(NH):
                qk_head(kpt, wk_v, k_sb, h)

            for jt in range(NJT):
                ps = ppsum.tile([128, NDV], F32, tag="pv", name="pv",
                                bufs=PV_BUFS)
                for ci in range(NVT):
                    nc.tensor.matmul(
                        ps,
                        lhsT=v_sb[:, ci, jt * 128:(jt + 1) * 128],
                        rhs=wv_v[:, ci, :],
                        start=(ci == 0), stop=(ci == NVT - 1),
                    )
                nc.vector.tensor_copy(out=vp[:, jt, :], in_=ps)

        # ---- attention ----
        # consecutive phases share ONE U.T PSUM tile through opposite
        # partition halves (the PE array column-offsets the AV matmuls via
        # tile_position), freeing two PSUM banks for a third S buffer
        spsum = top.enter_context(tc.tile_pool(name="spsum", bufs=3, space="PSUM"))
        utpsum = top.enter_context(tc.tile_pool(name="utpsum", bufs=1, space="PSUM"))
        streams = top.enter_context(tc.tile_pool(name="streams", bufs=3))
        utsb_pool = top.enter_context(tc.tile_pool(name="utsb", bufs=2))

        ut_all = utpsum.tile([128, IH], F32, tag="ut", name="ut")
        deferred = []              # per-phase tail closures

        for h in range(NH):
            for ihalf in range(NHF):
                i0 = ihalf * IH
                ph = h * NHF + ihalf
                ut_ps = ut_all[(ph % 2) * DV:(ph % 2) * DV + DV, :]
                eacc = streams.tile([128, 2, IH], BF16, tag="esum",
                                    name="eacc", bufs=2)
                av_emitted = [0]   # count of AV matmul groups written

                def emit_av(jt, bsb_half, ut_ps=ut_ps, h=h, av_emitted=av_emitted):
                    first = av_emitted[0] == 0
                    last = av_emitted[0] == NJT - 1
                    av_emitted[0] += 1
                    for ic in range(IH // 512):
                        sl = slice(ic * 512, (ic + 1) * 512)
                        nc.tensor.matmul(
                            ut_ps[:, sl],
                            lhsT=vp[:, jt, h * DV:(h + 1) * DV],
                            rhs=bsb_half[:, sl],
                            start=first, stop=last, skip_group_check=True,
                        )

                pending = []       # [(jt, bsb_tile)] AV deferred two j-tiles
                first_eacc = True
                eacc_pending = None   # expst awaiting its eacc add
                for p in range(NPR):
                    expst = streams.tile([128, 2, IH], BF16, tag="expst",
                                         name="expst", bufs=5)
                    for t in range(2):
                        jt = 2 * p + t
                        s_ps = spsum.tile([128, IH], F32, tag="s", name="s_ps")
                        for q2 in range(IH // 512):
                            nc.tensor.matmul(
                                s_ps[:, q2 * 512:(q2 + 1) * 512],
                                lhsT=kpt[h][:, jt * 128:(jt + 1) * 128],
                                rhs=qpt[h][:, i0 + q2 * 512:
                                           i0 + (q2 + 1) * 512],
                                start=True, stop=True,
                            )
                        if p == 0 and t == 1 and deferred:
                            # previous phase's PE tail — emitted after this
                            # phase's first two S blocks so the exp pipe
                            # restarts before the PE touches the old tail
                            deferred.pop(0)()
                        while pending and pending[0][0] <= jt - 3:
                            emit_av(*pending.pop(0))
                        nc.scalar.activation(
                            out=expst[:, t, :], in_=s_ps,
                            func=mybir.ActivationFunctionType.Exp,
                            scale=SCALE,
                        )
                        # per-j-tile mask multiply: half the latency of a
                        # pair op, so the AV matmuls are never starved
                        bsb = streams.tile([128, IH], BF16, tag="b",
                                           name="bsb", bufs=5)
                        nc.vector.tensor_tensor(
                            out=bsb, in0=expst[:, t, :],
                            in1=mt_all[:, jt, i0:i0 + IH],
                            op=mybir.AluOpType.mult)
                        pending.append((jt, bsb))
                    if p == 1 and deferred:
                        # previous phase's copy + output DMAs
                        deferred.pop(0)()
                    # eacc add of the PREVIOUS pair: the adds feed nothing
                    # urgent, so they trail the bsb ops on the DVE queue
                    if eacc_pending is not None:
                        if first_eacc:
                            nc.vector.tensor_copy(out=eacc, in_=eacc_pending)
                            first_eacc = False
                        else:
                            nc.vector.tensor_tensor(
                                out=eacc, in0=eacc, in1=eacc_pending,
                                op=mybir.AluOpType.add)
                        eacc_pending = None
                    # running per-partition exp sums; the last NRW pairs
                    # ship raw instead (host sums them — shorter DVE
                    # chain). Their DMAs go out immediately: the ring
                    # buffer is rewritten early next phase, and a DMA
                    # emitted after that write would read garbage.
                    if p >= NPR - NRW:
                        ti = p - (NPR - NRW) + 1
                        eo = (h * NHF + ihalf) * ESH
                        nc.sync.dma_start(
                            out=eout[:, eo + 2 * IH * ti:
                                     eo + 2 * IH * (ti + 1)],
                            in_=expst.rearrange("p a n -> p (a n)"))
                    else:
                        eacc_pending = expst
                if eacc_pending is not None:
                    nc.vector.tensor_tensor(
                        out=eacc, in0=eacc, in1=eacc_pending,
                        op=mybir.AluOpType.add)
                    eacc_pending = None

                def epilogue_pe(pending=list(pending), emit_av=emit_av):
                    for a in pending:
                        emit_av(*a)

                def epilogue_out(h=h, ihalf=ihalf, ut_ps=ut_ps, eacc=eacc):
                    # ship U.T (PSUM->SBUF copy on the scalar engine — the
                    # DVE is the pacing engine) and the exp-sum tile; the
                    # partition contraction, division and transpose all
                    # happen on the host
                    ut_sb = utsb_pool.tile([DV, IH], F32, tag="utsb",
                                           name="utsb")
                    nc.vector.tensor_copy(out=ut_sb, in_=ut_ps)
                    off = (h * NHF + ihalf) * IH
                    nc.sync.dma_start(out=out[:, off:off + IH], in_=ut_sb)
                    eo = (h * NHF + ihalf) * ESH
                    nc.sync.dma_start(
                        out=eout[:, eo:eo + 2 * IH],
                        in_=eacc.rearrange("p a n -> p (a n)"))

                deferred.append(epilogue_pe)
                deferred.append(epilogue_out)
        while deferred:
            deferred.pop(0)()

    nc.finalize()
    return nc


_NC_CACHE: dict = {}


def get_nc(NT: int = N):
    if NT not in _NC_CACHE:
        _NC_CACHE[NT] = build_nc(NT)
    return _NC_CACHE[NT]


def _pack(x):
    """[k*128, W] -> [128, k*W]: partition p holds rows {p, 128+p, ...}."""
    k = x.shape[0] // 128
    return x.reshape(k, 128, -1).transpose(1, 0, 2).reshape(128, -1)


def pack_core(qb, kb, vb, mb, wq_s, wk_s, wv_s):
    """Build one core's packed bf16 input dict from raw (transposed) slices."""

    def bf(x):
        return np.ascontiguousarray(_pack(x.astype(np.float32).astype(BF16NP)))

    wall = np.concatenate(
        [_pack(wq_s.astype(np.float32).astype(BF16NP)),
         _pack(wk_s.astype(np.float32).astype(BF16NP)),
         _pack(wv_s.astype(np.float32).astype(BF16NP))], axis=1)
    return {
        "qT": bf(qb), "kT": bf(kb), "vT": bf(vb), "mT": bf(mb),
        "wall": np.ascontiguousarray(wall),
    }


def make_in_maps(q, k, v, masks, Wq, Wk, Wv):
    """Host-side shard + layout prep. Returns per-core input dicts."""
    in_maps = []
    for c in range(N_CORES):
        b, hg = c // 2, c % 2
        in_maps.append(pack_core(
            q[b].T, k[b].T, v[b].T, masks[b].T,
            Wq[hg * NDO:(hg + 1) * NDO, :].T,
            Wk[hg * NDO:(hg + 1) * NDO, :].T,
            Wv[hg * NDV:(hg + 1) * NDV, :].T,
        ))
    return in_maps


def unshard(results, masks, NT=N):
    """Assemble full [B, N, CV] output from per-core U.T results.

    results[c]["out"] is [64, NH*NHF*IH]: per (head, i-half) chunks of
    U.T. results[c]["eout"] is [128, NH*NHF*(1+NRW)*2*IH]: the bf16
    exp-sum accumulator + raw pair tiles; contracting them over
    partitions/tiles here yields sumexp. x = U / (sumexp * 8 * summ).
    """
    IH = min(1024, NT)
    NHF = NT // IH
    NRW = min(RAW_PAIRS, NT // 256 - 1)
    summ8 = 8.0 * np.asarray(masks, np.float64).sum(-1)      # [B, N]
    full = np.empty((B, NT, CV), np.float32)
    for c, res in enumerate(results):
        b, hg = c // 2, c % 2
        ut = np.asarray(res["out"], np.float64).reshape(DV, NH, NT)
        ea = np.asarray(res["eout"], np.float32).reshape(
            128, NH, NHF, 1 + NRW, 2, IH)   # [p, h, ihalf, tile, half, i]
        sumexp = ea.sum(axis=(0, 3, 4)).reshape(NH, NT)      # [NH, N]
        den = sumexp * summ8[b][None, :]                     # [NH, N]
        x = ut / den[None, :, :]                             # [DV, NH, N]
        full[b][:, hg * NDV:(hg + 1) * NDV] = (
            x.transpose(2, 1, 0).reshape(NT, NDV))
    return full


def _reset_device():
    import ctypes
    try:
        lib = ctypes.CDLL("/opt/axon/libaxon_pjrt.so")
        lib.axon_reset.restype = ctypes.c_int64
        lib.axon_reset()
    except Exception:
        pass


def kernel(q, k, v, masks, Wq, Wk, Wv, **_unused):
    from concourse.bass_utils import run_bass_kernel_spmd

    q, k, v, masks = (np.asarray(x) for x in (q, k, v, masks))
    Wq, Wk, Wv = (np.asarray(x) for x in (Wq, Wk, Wv))

    nc = get_nc(N)
    in_maps = make_in_maps(q, k, v, masks, Wq, Wk, Wv)
    try:
        res = run_bass_kernel_spmd(
            nc, in_maps, core_ids=list(range(N_CORES))).results
    except Exception:
        # wedged accelerator (e.g. NRT_EXEC_UNIT_UNRECOVERABLE) — reset + retry
        _reset_device()
        res = run_bass_kernel_spmd(
            nc, in_maps, core_ids=list(range(N_CORES))).results

    return unshard(res, masks)
